# revision 15
# baseline (speedup 1.0000x reference)
"""Trainium2 Bass kernel for the grouped linear ensemble (moe_routing).

Problem: x [262144, 256] f32, Ws [64, 4, 256, 256], bs [64, 4, 256].
Model m applies its 4-layer stack (h = h @ W_l + b_l) to its contiguous
4096-row slice of x.

Sharding: expert parallel — core c owns models 8c..8c+7 and their rows.
No cross-device communication.

Per-core kernel design (v2 — fp16, transpose-free):
- The 4-layer chain is affine; the host composes it into a single layer
  per model (Wc = W1 W2 W3 W4, bc folded likewise, in float64).
- All device traffic is fp16 (tolerance is 2e-2; fp16 keeps rel err
  ~3e-4): x is pre-transposed AND pre-cast on the host so each core
  loads feature-major tiles [f=128 partitions, t free] directly — no
  on-chip transposes, no PSUM round-trip for activations.
- Chunks of 1024 rows stream through: one DMA load of x^T
  [128, 2 fblocks, 1024], then per 128-row j-block a pair of matmuls
  (lhsT = x^T block stationary, rhs = Wc fblock streaming 256 cols)
  accumulates y = x @ Wc in PSUM f32 in natural row-major orientation.
- The bias is pre-broadcast on the host to [128, 256] f32 per model and
  folded into the PSUM->SBUF output copy as a tensor_tensor add,
  alternating DVE and Pool so neither engine binds. Output is stored
  fp16 and upcast on the host.
"""

from contextlib import ExitStack

import numpy as np

import concourse.tile as tile
import concourse.mybir as mybir
from concourse import bacc
from concourse.bass_utils import run_bass_kernel_spmd

N_CORES = 8
N_MODELS = 64
N_LAYERS = 4
F = 256
ROWS_PER_MODEL = 4096
M_PER_CORE = N_MODELS // N_CORES          # 8 models per core
ROWS_PER_CORE = M_PER_CORE * ROWS_PER_MODEL  # 32768
CHUNK = 2048                              # rows of x processed per pipeline step
JG = CHUNK // 128                         # 128-row j-blocks per chunk
CHUNKS_PER_MODEL = ROWS_PER_MODEL // CHUNK   # 2

F32 = mybir.dt.float32
FP16 = mybir.dt.float16
COPY = mybir.ActivationFunctionType.Copy


def emit_core_kernel(tc, xT_d, wc_d, bb_d, bch_d, y_d, reps=1):
    nc = tc.nc

    ctx = ExitStack()
    const = ctx.enter_context(tc.tile_pool(name="const", bufs=1))
    wpool = ctx.enter_context(tc.tile_pool(name="w", bufs=2))
    hpool = ctx.enter_context(tc.tile_pool(name="h", bufs=3))
    opool = ctx.enter_context(tc.tile_pool(name="o", bufs=3))
    psL = ctx.enter_context(tc.tile_pool(name="psL", bufs=4, space="PSUM"))

    ones_f = const.tile([1, 128], F32)
    nc.gpsimd.memset(ones_f[:], 1.0)
    onesh = const.tile([1, 128], FP16)
    nc.vector.tensor_copy(onesh[:], ones_f[:])

    def body():
      for m in range(M_PER_CORE):
        # per-model composed weights + broadcast bias (double-buffered)
        wc = []
        for fb in range(2):
            wr = wpool.tile([128, F], FP16, tag=f"wr_{fb}")
            nc.sync.dma_start(wr[:], wc_d[m, fb * 128:(fb + 1) * 128, :])
            wc.append(wr)
        bb = wpool.tile([128, F], F32, tag="bb")
        nc.sync.dma_start(bb[:], bb_d[m])
        # bias doubled along j for the paired 2-block drains (built on Pool)
        bb2 = wpool.tile([128, 2, F], F32, tag="bb2")
        for q in range(2):
            nc.gpsimd.tensor_copy(bb2[:, q, :], bb[:])

        for c in range(CHUNKS_PER_MODEL):
            r0 = (m * CHUNKS_PER_MODEL + c) * CHUNK
            # x^T chunk: [128 (f within block), 2 (fblock), CHUNK (t)];
            # split across the two HWDGE rings (SP + ACT) for DMA throughput
            h = hpool.tile([128, 2, CHUNK], FP16, tag="h")
            xv = xT_d[:, r0:r0 + CHUNK].rearrange("(fb p) t -> p fb t", fb=2)
            nc.sync.dma_start(h[:, 0], xv[:, 0])
            nc.scalar.dma_start(h[:, 1], xv[:, 1])
            on = opool.tile([128, JG, F], FP16, tag="on")
            for jp in range(JG // 2):
                # paired j-blocks share one PSUM tile (one bank) so the DVE
                # drain amortizes its PSUM access latency over 512 rows
                p4 = psL.tile([128, 2, F], F32, tag="p4")
                for q in range(2):
                    j = jp * 2 + q
                    for fb in range(2):
                        nc.tensor.matmul(
                            p4[:, q, :],
                            h[:, fb, j * 128:(j + 1) * 128],
                            wc[fb][:],
                            start=(fb == 0),
                            stop=(fb == 1),
                        )
                # all drains on DVE (bias folded in); ACT stays DMA-only
                nc.vector.tensor_add(on[:, jp * 2:jp * 2 + 2, :], p4[:], bb2[:])
            # p-major row mapping (host pre-permuted x^T columns): partition
            # p holds rows p*JG..p*JG+JG-1 contiguous; store split across
            # all three DMA paths (SP + ACT HWDGE rings, Pool SWDGE)
            yv = y_d[r0:r0 + CHUNK, :].rearrange("(p j) f -> p j f", j=JG)
            nc.sync.dma_start(yv[:, 0:3], on[:, 0:3])
            nc.scalar.dma_start(yv[:, 3:7], on[:, 3:7])
            nc.gpsimd.dma_start(yv[:, 7:JG], on[:, 7:JG])

    if reps == 1:
        body()
    else:
        with tc.For_i(0, reps, 1):
            body()
    ctx.close()


def build_nc(reps=1):
    nc = bacc.Bacc("TRN2", target_bir_lowering=False, debug=False,
                   num_devices=N_CORES)
    xT_d = nc.dram_tensor("xT", [F, ROWS_PER_CORE], FP16,
                          kind="ExternalInput").ap()
    wc_d = nc.dram_tensor("Wc", [M_PER_CORE, F, F], FP16,
                          kind="ExternalInput").ap()
    bb_d = nc.dram_tensor("bb", [M_PER_CORE, 128, F], F32,
                          kind="ExternalInput").ap()
    bch_d = nc.dram_tensor("bch", [M_PER_CORE, 1, F], FP16,
                           kind="ExternalInput").ap()
    y_d = nc.dram_tensor("y", [ROWS_PER_CORE, F], FP16,
                         kind="ExternalOutput").ap()
    with tile.TileContext(nc) as tc:
        emit_core_kernel(tc, xT_d, wc_d, bb_d, bch_d, y_d, reps=reps)
    nc.compile()
    return nc


_NC = None


def _get_nc():
    global _NC
    if _NC is None:
        _NC = build_nc()
    return _NC


def _compose_affine(Ws, bs):
    """Fold the 4-layer affine chain into one layer per model (float64)."""
    W = np.asarray(Ws, dtype=np.float64)
    b = np.asarray(bs, dtype=np.float64)
    Wc = W[:, 0]
    bc = b[:, 0]
    for l in range(1, N_LAYERS):
        Wc = np.matmul(Wc, W[:, l])
        bc = np.matmul(bc[:, None, :], W[:, l])[:, 0] + b[:, l]
    return Wc, bc


def make_in_maps(x, Ws, bs):
    Wc, bc = _compose_affine(Ws, bs)
    Wch = Wc.astype(np.float16)
    # broadcast bias to [128, F] f32 per model (added during PSUM drain)
    bb = np.ascontiguousarray(
        np.broadcast_to(bc.astype(np.float32)[:, None, :], (N_MODELS, 128, F)))
    xh = np.asarray(x, dtype=np.float16)
    in_maps = []
    for c in range(N_CORES):
        m0 = c * M_PER_CORE
        r0 = m0 * ROWS_PER_MODEL
        # feature-major x with chunk-local column permutation t' = j*128 + p
        # holding original row p*JG + j: j-block j's matmul then emits rows
        # {p*JG + j}, so partition p's store run is contiguous in DRAM.
        xc = xh[r0:r0 + ROWS_PER_CORE].reshape(-1, 128, JG, F)
        xTp = np.ascontiguousarray(
            xc.transpose(3, 0, 2, 1).reshape(F, ROWS_PER_CORE))
        in_maps.append({
            "xT": xTp,
            "Wc": np.ascontiguousarray(Wch[m0:m0 + M_PER_CORE]),
            "bb": np.ascontiguousarray(bb[m0:m0 + M_PER_CORE]),
            "bch": np.ascontiguousarray(
                bc.astype(np.float16)[m0:m0 + M_PER_CORE, None, :]),
        })
    return in_maps


def kernel(x, Ws, bs, slice_bounds=None, **_):
    x = np.asarray(x, dtype=np.float32)
    Ws = np.asarray(Ws, dtype=np.float32)
    bs = np.asarray(bs, dtype=np.float32)
    nc = _get_nc()
    res = run_bass_kernel_spmd(nc, make_in_maps(x, Ws, bs),
                               core_ids=list(range(N_CORES)))
    return np.concatenate(
        [res.results[c]["y"].astype(np.float32) for c in range(N_CORES)], axis=0)


# revision 16
# speedup vs baseline: 1.0066x; 1.0066x over previous
"""Trainium2 Bass kernel for the grouped linear ensemble (moe_routing).

Problem: x [262144, 256] f32, Ws [64, 4, 256, 256], bs [64, 4, 256].
Model m applies its 4-layer stack (h = h @ W_l + b_l) to its contiguous
4096-row slice of x.

Sharding: expert parallel — core c owns models 8c..8c+7 and their rows.
No cross-device communication.

Per-core kernel design (v2 — fp16, transpose-free):
- The 4-layer chain is affine; the host composes it into a single layer
  per model (Wc = W1 W2 W3 W4, bc folded likewise, in float64).
- All device traffic is fp16 (tolerance is 2e-2; fp16 keeps rel err
  ~3e-4): x is pre-transposed AND pre-cast on the host so each core
  loads feature-major tiles [f=128 partitions, t free] directly — no
  on-chip transposes, no PSUM round-trip for activations.
- Chunks of 1024 rows stream through: one DMA load of x^T
  [128, 2 fblocks, 1024], then per 128-row j-block a pair of matmuls
  (lhsT = x^T block stationary, rhs = Wc fblock streaming 256 cols)
  accumulates y = x @ Wc in PSUM f32 in natural row-major orientation.
- The bias is pre-broadcast on the host to [128, 256] f32 per model and
  folded into the PSUM->SBUF output copy as a tensor_tensor add,
  alternating DVE and Pool so neither engine binds. Output is stored
  fp16 and upcast on the host.
"""

from contextlib import ExitStack

import numpy as np

import concourse.tile as tile
import concourse.mybir as mybir
from concourse import bacc
from concourse.bass_utils import run_bass_kernel_spmd

N_CORES = 8
N_MODELS = 64
N_LAYERS = 4
F = 256
ROWS_PER_MODEL = 4096
M_PER_CORE = N_MODELS // N_CORES          # 8 models per core
ROWS_PER_CORE = M_PER_CORE * ROWS_PER_MODEL  # 32768
CHUNK = 2048                              # rows of x processed per pipeline step
JG = CHUNK // 128                         # 128-row j-blocks per chunk
CHUNKS_PER_MODEL = ROWS_PER_MODEL // CHUNK   # 2

F32 = mybir.dt.float32
FP16 = mybir.dt.float16
COPY = mybir.ActivationFunctionType.Copy


def emit_core_kernel(tc, xT_d, wc_d, bb_d, bch_d, y_d, reps=1):
    nc = tc.nc

    ctx = ExitStack()
    const = ctx.enter_context(tc.tile_pool(name="const", bufs=1))
    wpool = ctx.enter_context(tc.tile_pool(name="w", bufs=2))
    hpool = ctx.enter_context(tc.tile_pool(name="h", bufs=3))
    opool = ctx.enter_context(tc.tile_pool(name="o", bufs=3))
    psL = ctx.enter_context(tc.tile_pool(name="psL", bufs=4, space="PSUM"))

    ones_f = const.tile([1, 128], F32)
    nc.gpsimd.memset(ones_f[:], 1.0)
    onesh = const.tile([1, 128], FP16)
    nc.vector.tensor_copy(onesh[:], ones_f[:])

    def body():
      for m in range(M_PER_CORE):
        # per-model composed weights + broadcast bias (double-buffered)
        wc = []
        for fb in range(2):
            wr = wpool.tile([128, F], FP16, tag=f"wr_{fb}")
            nc.sync.dma_start(wr[:], wc_d[m, fb * 128:(fb + 1) * 128, :])
            wc.append(wr)
        bb = wpool.tile([128, F], F32, tag="bb")
        nc.sync.dma_start(bb[:], bb_d[m])
        # bias doubled along j for the paired 2-block drains (built on Pool)
        bb2 = wpool.tile([128, 2, F], F32, tag="bb2")
        for q in range(2):
            nc.gpsimd.tensor_copy(bb2[:, q, :], bb[:])

        for c in range(CHUNKS_PER_MODEL):
            r0 = (m * CHUNKS_PER_MODEL + c) * CHUNK
            # x^T chunk: [128 (f within block), 2 (fblock), CHUNK (t)];
            # split across the two HWDGE rings (SP + ACT) for DMA throughput
            h = hpool.tile([128, 2, CHUNK], FP16, tag="h")
            xv = xT_d[:, r0:r0 + CHUNK].rearrange("(fb p) t -> p fb t", fb=2)
            nc.sync.dma_start(h[:, 0], xv[:, 0])
            nc.scalar.dma_start(h[:, 1], xv[:, 1])
            on = opool.tile([128, JG, F], FP16, tag="on")
            for jp in range(JG // 2):
                # paired j-blocks share one PSUM tile (one bank) so the DVE
                # drain amortizes its PSUM access latency over 512 rows
                p4 = psL.tile([128, 2, F], F32, tag="p4")
                for q in range(2):
                    j = jp * 2 + q
                    for fb in range(2):
                        nc.tensor.matmul(
                            p4[:, q, :],
                            h[:, fb, j * 128:(j + 1) * 128],
                            wc[fb][:],
                            start=(fb == 0),
                            stop=(fb == 1),
                        )
                # all drains on DVE (bias folded in); ACT stays DMA-only
                nc.vector.tensor_add(on[:, jp * 2:jp * 2 + 2, :], p4[:], bb2[:])
            # p-major row mapping (host pre-permuted x^T columns): partition
            # p holds rows p*JG..p*JG+JG-1 contiguous; store split across
            # all three DMA paths (SP + ACT HWDGE rings, Pool SWDGE)
            yv = y_d[r0:r0 + CHUNK, :].rearrange("(p j) f -> p j f", j=JG)
            jh = JG // 2
            nc.sync.dma_start(yv[:, :jh], on[:, :jh])
            nc.scalar.dma_start(yv[:, jh:], on[:, jh:])

    if reps == 1:
        body()
    else:
        with tc.For_i(0, reps, 1):
            body()
    ctx.close()


def build_nc(reps=1):
    nc = bacc.Bacc("TRN2", target_bir_lowering=False, debug=False,
                   num_devices=N_CORES)
    xT_d = nc.dram_tensor("xT", [F, ROWS_PER_CORE], FP16,
                          kind="ExternalInput").ap()
    wc_d = nc.dram_tensor("Wc", [M_PER_CORE, F, F], FP16,
                          kind="ExternalInput").ap()
    bb_d = nc.dram_tensor("bb", [M_PER_CORE, 128, F], F32,
                          kind="ExternalInput").ap()
    bch_d = nc.dram_tensor("bch", [M_PER_CORE, 1, F], FP16,
                           kind="ExternalInput").ap()
    y_d = nc.dram_tensor("y", [ROWS_PER_CORE, F], FP16,
                         kind="ExternalOutput").ap()
    with tile.TileContext(nc) as tc:
        emit_core_kernel(tc, xT_d, wc_d, bb_d, bch_d, y_d, reps=reps)
    nc.compile()
    return nc


_NC = None


def _get_nc():
    global _NC
    if _NC is None:
        _NC = build_nc()
    return _NC


def _compose_affine(Ws, bs):
    """Fold the 4-layer affine chain into one layer per model (float64)."""
    W = np.asarray(Ws, dtype=np.float64)
    b = np.asarray(bs, dtype=np.float64)
    Wc = W[:, 0]
    bc = b[:, 0]
    for l in range(1, N_LAYERS):
        Wc = np.matmul(Wc, W[:, l])
        bc = np.matmul(bc[:, None, :], W[:, l])[:, 0] + b[:, l]
    return Wc, bc


def make_in_maps(x, Ws, bs):
    Wc, bc = _compose_affine(Ws, bs)
    Wch = Wc.astype(np.float16)
    # broadcast bias to [128, F] f32 per model (added during PSUM drain)
    bb = np.ascontiguousarray(
        np.broadcast_to(bc.astype(np.float32)[:, None, :], (N_MODELS, 128, F)))
    xh = np.asarray(x, dtype=np.float16)
    in_maps = []
    for c in range(N_CORES):
        m0 = c * M_PER_CORE
        r0 = m0 * ROWS_PER_MODEL
        # feature-major x with chunk-local column permutation t' = j*128 + p
        # holding original row p*JG + j: j-block j's matmul then emits rows
        # {p*JG + j}, so partition p's store run is contiguous in DRAM.
        xc = xh[r0:r0 + ROWS_PER_CORE].reshape(-1, 128, JG, F)
        xTp = np.ascontiguousarray(
            xc.transpose(3, 0, 2, 1).reshape(F, ROWS_PER_CORE))
        in_maps.append({
            "xT": xTp,
            "Wc": np.ascontiguousarray(Wch[m0:m0 + M_PER_CORE]),
            "bb": np.ascontiguousarray(bb[m0:m0 + M_PER_CORE]),
            "bch": np.ascontiguousarray(
                bc.astype(np.float16)[m0:m0 + M_PER_CORE, None, :]),
        })
    return in_maps


def kernel(x, Ws, bs, slice_bounds=None, **_):
    x = np.asarray(x, dtype=np.float32)
    Ws = np.asarray(Ws, dtype=np.float32)
    bs = np.asarray(bs, dtype=np.float32)
    nc = _get_nc()
    res = run_bass_kernel_spmd(nc, make_in_maps(x, Ws, bs),
                               core_ids=list(range(N_CORES)))
    return np.concatenate(
        [res.results[c]["y"].astype(np.float32) for c in range(N_CORES)], axis=0)


# revision 19
# speedup vs baseline: 1.1265x; 1.1191x over previous
"""Trainium2 Bass kernel for the grouped linear ensemble (moe_routing).

Problem: x [262144, 256] f32, Ws [64, 4, 256, 256], bs [64, 4, 256].
Model m applies its 4-layer stack (h = h @ W_l + b_l) to its contiguous
4096-row slice of x.

Sharding: expert parallel — core c owns models 8c..8c+7 and their rows.
No cross-device communication.

Per-core kernel design (v2 — fp16, transpose-free):
- The 4-layer chain is affine; the host composes it into a single layer
  per model (Wc = W1 W2 W3 W4, bc folded likewise, in float64).
- All device traffic is fp16 (tolerance is 2e-2; fp16 keeps rel err
  ~3e-4): x is pre-transposed AND pre-cast on the host so each core
  loads feature-major tiles [f=128 partitions, t free] directly — no
  on-chip transposes, no PSUM round-trip for activations.
- Chunks of 1024 rows stream through: one DMA load of x^T
  [128, 2 fblocks, 1024], then per 128-row j-block a pair of matmuls
  (lhsT = x^T block stationary, rhs = Wc fblock streaming 256 cols)
  accumulates y = x @ Wc in PSUM f32 in natural row-major orientation.
- The bias is pre-broadcast on the host to [128, 256] f32 per model and
  folded into the PSUM->SBUF output copy as a tensor_tensor add,
  alternating DVE and Pool so neither engine binds. Output is stored
  fp16 and upcast on the host.
"""

from contextlib import ExitStack

import numpy as np

import concourse.tile as tile
import concourse.mybir as mybir
from concourse import bacc
from concourse.bass_utils import run_bass_kernel_spmd

N_CORES = 8
N_MODELS = 64
N_LAYERS = 4
F = 256
ROWS_PER_MODEL = 4096
M_PER_CORE = N_MODELS // N_CORES          # 8 models per core
ROWS_PER_CORE = M_PER_CORE * ROWS_PER_MODEL  # 32768
CHUNK = 2048                              # rows of x processed per pipeline step
JG = CHUNK // 128                         # 128-row j-blocks per chunk
CHUNKS_PER_MODEL = ROWS_PER_MODEL // CHUNK   # 2

F32 = mybir.dt.float32
FP16 = mybir.dt.float16
COPY = mybir.ActivationFunctionType.Copy


def emit_core_kernel(tc, xT_d, wc_d, bb_d, bch_d, y_d, reps=1):
    nc = tc.nc

    ctx = ExitStack()
    const = ctx.enter_context(tc.tile_pool(name="const", bufs=1))
    wpool = ctx.enter_context(tc.tile_pool(name="w", bufs=2))
    hpool = ctx.enter_context(tc.tile_pool(name="h", bufs=3))
    opool = ctx.enter_context(tc.tile_pool(name="o", bufs=3))
    psL = ctx.enter_context(tc.tile_pool(name="psL", bufs=4, space="PSUM"))

    ones_f = const.tile([1, 128], F32)
    nc.gpsimd.memset(ones_f[:], 1.0)
    onesh = const.tile([1, 128], FP16)
    nc.vector.tensor_copy(onesh[:], ones_f[:])

    def body():
      for m in range(M_PER_CORE):
        # per-model composed weights + broadcast bias (double-buffered)
        wc = []
        for fb in range(2):
            wr = wpool.tile([128, F], FP16, tag=f"wr_{fb}")
            nc.sync.dma_start(wr[:], wc_d[m, fb * 128:(fb + 1) * 128, :])
            wc.append(wr)
        bb = wpool.tile([128, F], F32, tag="bb")
        nc.sync.dma_start(bb[:], bb_d[m])

        for c in range(CHUNKS_PER_MODEL):
            r0 = (m * CHUNKS_PER_MODEL + c) * CHUNK
            # x^T chunk: [128 (f within block), 2 (fblock), CHUNK (t)];
            # split across the two HWDGE rings (SP + ACT) for DMA throughput
            h = hpool.tile([128, 2, CHUNK], FP16, tag="h")
            xv = xT_d[:, r0:r0 + CHUNK].rearrange("(fb p) t -> p fb t", fb=2)
            nc.sync.dma_start(h[:, 0], xv[:, 0])
            nc.scalar.dma_start(h[:, 1], xv[:, 1])
            on = opool.tile([128, JG, F], FP16, tag="on")
            for j in range(JG):
                p4 = psL.tile([128, F], F32, tag="p4")
                for fb in range(2):
                    nc.tensor.matmul(
                        p4[:],
                        h[:, fb, j * 128:(j + 1) * 128],
                        wc[fb][:],
                        start=(fb == 0),
                        stop=(fb == 1),
                    )
                # all drains on DVE (bias folded in); ACT stays DMA-only
                nc.vector.tensor_add(on[:, j, :], p4[:], bb[:])
            # p-major row mapping (host pre-permuted x^T columns): partition
            # p holds rows p*JG..p*JG+JG-1 contiguous; store split across
            # all three DMA paths (SP + ACT HWDGE rings, Pool SWDGE)
            # three DMA paths: SP + ACT HWDGE rings carry most, Pool SWDGE
            # takes a slice of the store to offload the HWDGE rings
            yv = y_d[r0:r0 + CHUNK, :].rearrange("(p j) f -> p j f", j=JG)
            nc.sync.dma_start(yv[:, 0:5], on[:, 0:5])
            nc.scalar.dma_start(yv[:, 5:11], on[:, 5:11])
            nc.gpsimd.dma_start(yv[:, 11:JG], on[:, 11:JG])

    if reps == 1:
        body()
    else:
        with tc.For_i(0, reps, 1):
            body()
    ctx.close()


def build_nc(reps=1):
    nc = bacc.Bacc("TRN2", target_bir_lowering=False, debug=False,
                   num_devices=N_CORES)
    xT_d = nc.dram_tensor("xT", [F, ROWS_PER_CORE], FP16,
                          kind="ExternalInput").ap()
    wc_d = nc.dram_tensor("Wc", [M_PER_CORE, F, F], FP16,
                          kind="ExternalInput").ap()
    bb_d = nc.dram_tensor("bb", [M_PER_CORE, 128, F], F32,
                          kind="ExternalInput").ap()
    bch_d = nc.dram_tensor("bch", [M_PER_CORE, 1, F], FP16,
                           kind="ExternalInput").ap()
    y_d = nc.dram_tensor("y", [ROWS_PER_CORE, F], FP16,
                         kind="ExternalOutput").ap()
    with tile.TileContext(nc) as tc:
        emit_core_kernel(tc, xT_d, wc_d, bb_d, bch_d, y_d, reps=reps)
    nc.compile()
    return nc


_NC = None


def _get_nc():
    global _NC
    if _NC is None:
        _NC = build_nc()
    return _NC


def _compose_affine(Ws, bs):
    """Fold the 4-layer affine chain into one layer per model (float64)."""
    W = np.asarray(Ws, dtype=np.float64)
    b = np.asarray(bs, dtype=np.float64)
    Wc = W[:, 0]
    bc = b[:, 0]
    for l in range(1, N_LAYERS):
        Wc = np.matmul(Wc, W[:, l])
        bc = np.matmul(bc[:, None, :], W[:, l])[:, 0] + b[:, l]
    return Wc, bc


def make_in_maps(x, Ws, bs):
    Wc, bc = _compose_affine(Ws, bs)
    Wch = Wc.astype(np.float16)
    # broadcast bias to [128, F] f32 per model (added during PSUM drain)
    bb = np.ascontiguousarray(
        np.broadcast_to(bc.astype(np.float32)[:, None, :], (N_MODELS, 128, F)))
    xh = np.asarray(x, dtype=np.float16)
    in_maps = []
    for c in range(N_CORES):
        m0 = c * M_PER_CORE
        r0 = m0 * ROWS_PER_MODEL
        # feature-major x with chunk-local column permutation t' = j*128 + p
        # holding original row p*JG + j: j-block j's matmul then emits rows
        # {p*JG + j}, so partition p's store run is contiguous in DRAM.
        xc = xh[r0:r0 + ROWS_PER_CORE].reshape(-1, 128, JG, F)
        xTp = np.ascontiguousarray(
            xc.transpose(3, 0, 2, 1).reshape(F, ROWS_PER_CORE))
        in_maps.append({
            "xT": xTp,
            "Wc": np.ascontiguousarray(Wch[m0:m0 + M_PER_CORE]),
            "bb": np.ascontiguousarray(bb[m0:m0 + M_PER_CORE]),
            "bch": np.ascontiguousarray(
                bc.astype(np.float16)[m0:m0 + M_PER_CORE, None, :]),
        })
    return in_maps


def kernel(x, Ws, bs, slice_bounds=None, **_):
    x = np.asarray(x, dtype=np.float32)
    Ws = np.asarray(Ws, dtype=np.float32)
    bs = np.asarray(bs, dtype=np.float32)
    nc = _get_nc()
    res = run_bass_kernel_spmd(nc, make_in_maps(x, Ws, bs),
                               core_ids=list(range(N_CORES)))
    return np.concatenate(
        [res.results[c]["y"].astype(np.float32) for c in range(N_CORES)], axis=0)
